# revision 1
# baseline (speedup 1.0000x reference)
"""Trainium2 Bass kernel for nn_AttentionSeparateQKV (B=16, N=1024, D=768, H=12).

Data-parallel over batch: 8 NeuronCores x 2 batches each. Per core:
  x arrives host-pretransposed (feature-major xT); weights host-pretransposed
  per feature-tile f (= head pair 2f, 2f+1), software-pipelined:
    qT/kT projections (fp32r matmuls, bias fused in PSUM->SBUF copy)
    v slice via XBAR DMA transpose of bf16 k
    scores[q,k] = qT_h^T @ kT_h  (head-pair row-packed, K=64, interleaved)
    exp on ScalarE (scale fused, accum_out -> softmax denominators), probs bf16
    probs -> probsT via XBAR DMA transpose (bf16)
    AV: outT pair = v^T-form matmul with probsT (head-pair col-packed)
  normalize via selector-matmul-replicated 1/denom; out-proj fp32r + bias matmul
"""

import sys

if "/opt/trn_rl_repo" not in sys.path:
    sys.path.insert(0, "/opt/trn_rl_repo")

import numpy as np

B, N, D, H = 16, 1024, 768, 12
HD = D // H                # 64
SCALE = float(HD) ** -0.5  # 0.125
N_CORES = 8
BL = B // N_CORES          # 2 batches per core
T = BL * N                 # 2048 tokens per core
FT = D // 128              # 6 feature tiles == head pairs
NQT = N // 128             # 8 query tiles per batch
QC = 512                   # query chunk (attention granularity)
NCH = N // QC              # 2 chunks per batch
QTC = QC // 128            # 4 query tiles per chunk

_NC_CACHE = []


def _build():
    import concourse.mybir as mybir
    import concourse.tile as tile
    from concourse import bacc

    F32 = mybir.dt.float32
    F32R = mybir.dt.float32r
    BF16 = mybir.dt.bfloat16
    EXP = mybir.ActivationFunctionType.Exp
    MULT = mybir.AluOpType.mult

    nc = bacc.Bacc("TRN2", target_bir_lowering=False, debug=False)

    x_d = nc.dram_tensor("x", [D, T], F32R, kind="ExternalInput").ap()  # host-pretransposed
    wq_d = nc.dram_tensor("wqt", [D, D], F32R, kind="ExternalInput").ap()
    wk_d = nc.dram_tensor("wkt", [D, D], F32R, kind="ExternalInput").ap()
    wp_d = nc.dram_tensor("wpt", [D, D], F32R, kind="ExternalInput").ap()
    bq_d = nc.dram_tensor("bqp", [128, FT], F32, kind="ExternalInput").ap()
    bk_d = nc.dram_tensor("bkp", [128, FT], F32, kind="ExternalInput").ap()
    bc_d = nc.dram_tensor("bc", [33, 1024], F32R, kind="ExternalInput").ap()
    id_d = nc.dram_tensor("ident", [128, 128], F32, kind="ExternalInput").ap()
    out_d = nc.dram_tensor("out", [T, D], F32, kind="ExternalOutput").ap()

    with tile.TileContext(nc) as tc:
        with (
            tc.tile_pool(name="const", bufs=1) as cpool,
            tc.tile_pool(name="perb", bufs=1) as bpool,
            tc.tile_pool(name="proj", bufs=3) as proj_pool,
            tc.tile_pool(name="probs", bufs=6) as probs_pool,
            tc.tile_pool(name="probsT", bufs=5) as pT_pool,
            tc.tile_pool(name="dn", bufs=2) as dn_pool,
            tc.tile_pool(name="outTp", bufs=1) as outT_pool,
            tc.tile_pool(name="fin", bufs=3) as fin_pool,
            tc.tile_pool(name="ps_big", bufs=2, space="PSUM") as ps_big,
            tc.tile_pool(name="ps_av", bufs=2, space="PSUM") as ps_av,
            tc.tile_pool(name="ps_misc", bufs=2, space="PSUM") as ps_misc,
        ):
            # ---- small constants needed immediately ----
            id_sb = cpool.tile([128, 128], F32, tag="ident")
            nc.scalar.dma_start(id_sb[:], id_d[:])
            bq_sb = cpool.tile([128, FT], F32, tag="bq")
            nc.scalar.dma_start(bq_sb[:], bq_d[:])
            bk_sb = cpool.tile([128, FT], F32, tag="bk")
            nc.scalar.dma_start(bk_sb[:], bk_d[:])

            def emit_xT(b):
                """Load feature-major x slice for batch b (host-pretransposed)."""
                xT = bpool.tile([128, FT, N], F32R, tag="xT")
                nc.gpsimd.dma_start(
                    xT[:],
                    x_d[:, b * N : (b + 1) * N].rearrange("(ko kp) t -> kp ko t", kp=128),
                )
                return xT

            def emit_proj(xT, f):
                """Q/K projections for feature tile f; returns (qTf, kTf, vf)."""
                qTf = proj_pool.tile([128, N], F32R, tag="qTf")
                kTf = proj_pool.tile([128, N], F32R, tag="kTf")
                vf = proj_pool.tile([128, NQT, 128], BF16, tag="vf")
                k16 = probs_pool.tile([128, N], BF16, tag="probs")
                for qs in range(N // 512):
                    pq = ps_misc.tile([128, 512], F32, tag="misc")
                    for ks in range(FT):
                        nc.tensor.matmul(
                            pq[:],
                            wq_sb[:, ks, 128 * f : 128 * (f + 1)],
                            xT[:, ks, 512 * qs : 512 * (qs + 1)],
                            start=(ks == 0),
                            stop=(ks == FT - 1),
                        )
                    nc.vector.tensor_scalar_add(
                        qTf[:, 512 * qs : 512 * (qs + 1)], pq[:], bq_sb[:, f : f + 1]
                    )
                    pk = ps_misc.tile([128, 512], F32, tag="misc")
                    for ks in range(FT):
                        nc.tensor.matmul(
                            pk[:],
                            wk_sb[:, ks, 128 * f : 128 * (f + 1)],
                            xT[:, ks, 512 * qs : 512 * (qs + 1)],
                            start=(ks == 0),
                            stop=(ks == FT - 1),
                        )
                    nc.vector.tensor_scalar_add(
                        kTf[:, 512 * qs : 512 * (qs + 1)], pk[:], bk_sb[:, f : f + 1]
                    )
                    nc.vector.tensor_scalar_add(
                        k16[:, 512 * qs : 512 * (qs + 1)], pk[:], bk_sb[:, f : f + 1]
                    )
                nc.sync.dma_start(vf[:], k16[:], transpose=True)
                return qTf, kTf, vf

            def emit_normalize(outT, denom, c):
                """Chunk-c softmax normalization of outT via replicated 1/denom."""
                recip = dn_pool.tile([128, QTC * H], F32, tag="recip")
                nc.vector.reciprocal(recip[:], denom[:])
                recipT = dn_pool.tile([H, QTC, 128], F32R, tag="recipT")
                for ql in range(QTC):
                    pt = ps_misc.tile([128, 512], F32, tag="misc")
                    nc.tensor.transpose(
                        pt[0:H, 0:128], recip[:, ql * H : (ql + 1) * H], id_sb[:]
                    )
                    nc.vector.tensor_copy(recipT[:, ql, :], pt[0:H, 0:128])
                for f in range(FT):
                    rp = ps_misc.tile([128, 512], F32, tag="misc")
                    nc.tensor.matmul(
                        rp[:], bc_sb[0:H, 128 * f : 128 * (f + 1)], recipT[:], start=True, stop=True
                    )
                    nc.vector.tensor_tensor(
                        outT[:, f, QC * c : QC * (c + 1)],
                        outT[:, f, QC * c : QC * (c + 1)],
                        rp[:],
                        MULT,
                    )

            def emit_outproj(outT, b, tts):
                """Final projection + bias for token tiles `tts` of batch b."""
                tok0 = b * N
                for tt in tts:
                    fin = fin_pool.tile([128, D], F32, tag="fin")
                    for ns in range(2):
                        pf = ps_misc.tile([128, 512], F32, tag="misc")
                        for ks in range(FT):
                            nc.tensor.matmul(
                                pf[:, 0:384],
                                outT[:, ks, 128 * tt : 128 * (tt + 1)],
                                wp_sb[:, ks, 384 * ns : 384 * (ns + 1)],
                                start=(ks == 0),
                                stop=False,
                            )
                        nc.tensor.matmul(
                            pf[:, 0:384],
                            bc_sb[32:33, 768:896],
                            bc_sb[32:33, 384 * ns : 384 * (ns + 1)],
                            start=False,
                            stop=True,
                        )
                        nc.vector.tensor_copy(fin[:, 384 * ns : 384 * (ns + 1)], pf[:, 0:384])
                    nc.scalar.dma_start(
                        out_d[tok0 + 128 * tt : tok0 + 128 * (tt + 1), :], fin[:]
                    )

            pending = []  # deferred out-projection pieces
            xT = emit_xT(0)
            wq_sb = cpool.tile([128, FT, D], F32R, tag="wq")
            wk_sb = cpool.tile([128, FT, D], F32R, tag="wk")
            wq_r = wq_d.rearrange("(ko kp) m -> kp ko m", kp=128)
            wk_r = wk_d.rearrange("(ko kp) m -> kp ko m", kp=128)
            for wf in range(FT):
                sl = slice(128 * wf, 128 * (wf + 1))
                nc.scalar.dma_start(wq_sb[:, :, sl], wq_r[:, :, sl])
                nc.scalar.dma_start(wk_sb[:, :, sl], wk_r[:, :, sl])
            nxt = emit_proj(xT, 0)
            wp_sb = cpool.tile([128, FT, D], F32R, tag="wp")
            nc.scalar.dma_start(wp_sb[:], wp_d.rearrange("(ko kp) m -> kp ko m", kp=128))
            bc_sb = cpool.tile([33, 1024], F32R, tag="bc")
            nc.scalar.dma_start(bc_sb[:], bc_d[:])
            xT_next = None
            pend_av = None

            def flush_av(outT, vf, pT_tiles, denoms, f, c, b):
                """Emit the deferred AV matmuls + outT copy for stage (b,f,c),
                plus per-chunk normalization and out-proj scheduling."""
                av = ps_av.tile([128, QC], F32, tag="av")
                for ks in range(NQT):
                    for e in range(2):
                        nc.tensor.matmul(
                            av[64 * e : 64 * (e + 1), :],
                            vf[:, ks, 64 * e : 64 * (e + 1)],
                            pT_tiles[e][:, :, ks, :],
                            start=(ks == 0),
                            stop=(ks == NQT - 1),
                        )
                nc.vector.tensor_copy(outT[:, f, QC * c : QC * (c + 1)], av[:])
                if f == FT - 1:
                    emit_normalize(outT, denoms[c], c)
                    pending.extend(
                        (outT, b, range(c * QTC + 2 * i, c * QTC + 2 * i + 2))
                        for i in range(QTC // 2)
                    )

            for b in range(BL):
                outT = outT_pool.tile([128, FT, N], F32R, tag="outT")
                denoms = [
                    dn_pool.tile([128, QTC * H], F32, tag="denom", name="denom")
                    for _ in range(NCH)
                ]
                for f in range(FT):
                    qTf, kTf, vf = nxt
                    for c in range(NCH):
                        denom = denoms[c]
                        pT_tiles = {}
                        for e in range(2):
                            pT_tiles[e] = pT_pool.tile(
                                [128, QTC, NQT, 128], BF16, tag="probsT", name="pT"
                            )
                        for ql in range(QTC):
                            qt = c * QTC + ql
                            sc = {}
                            for e in range(2):
                                sc[e] = ps_big.tile([128, N], F32, tag="sc", name="sc")
                            for kt in range(N // 512):
                                for e in range(2):
                                    nc.tensor.matmul(
                                        sc[e][:, 512 * kt : 512 * (kt + 1)],
                                        qTf[64 * e : 64 * (e + 1), 128 * qt : 128 * (qt + 1)],
                                        kTf[64 * e : 64 * (e + 1), 512 * kt : 512 * (kt + 1)],
                                        start=True,
                                        stop=True,
                                    )
                            for e in range(2):
                                h = 2 * f + e
                                pr = probs_pool.tile([128, N], BF16, tag="probs")
                                nc.scalar.activation(
                                    pr[:], sc[e][:], EXP, scale=SCALE,
                                    accum_out=denom[:, ql * H + h : ql * H + h + 1],
                                )
                                nc.sync.dma_start(
                                    pT_tiles[e][:, ql, :, :],
                                    pr[:],
                                    transpose=True,
                                )
                        # PE filler work emitted while ACT/DMA chew exp+transpose:
                        # next projections, the next batch's xT load, and the
                        # previous rounds' out-projection in 2-token-tile pieces.
                        if c == 0:
                            if f + 1 < FT:
                                nxt = emit_proj(xT, f + 1)
                        else:
                            if f == FT - 2 and b + 1 < BL:
                                xT_next = emit_xT(b + 1)
                            if f == FT - 1 and b + 1 < BL:
                                xT = xT_next
                                nxt = emit_proj(xT, 0)
                        # AV of the PREVIOUS stage (deferred so this stage's
                        # scores/exp issue before PE blocks on the previous
                        # stage's probs transposes)
                        if pend_av is not None:
                            flush_av(*pend_av)
                        for _ in range(2):
                            if pending:
                                emit_outproj(*pending.pop(0))
                        pend_av = (outT, vf, pT_tiles, denoms, f, c, b)
            flush_av(*pend_av)
            while pending:
                emit_outproj(*pending.pop(0))

    nc.compile()
    return nc


def _get_nc():
    if not _NC_CACHE:
        _NC_CACHE.append(_build())
    return _NC_CACHE[0]


def _to_np(a):
    try:
        return np.asarray(a)
    except Exception:
        import jax

        return np.asarray(jax.device_get(a))


def _prep_inputs(x, Wq, bq, Wk, bk, Wp, bp):
    x, Wq, bq, Wk, bk, Wp, bp = (
        _to_np(a) for a in (x, Wq, bq, Wk, bk, Wp, bp)
    )
    x = np.ascontiguousarray(np.asarray(x, dtype=np.float32))
    wqt = np.ascontiguousarray(np.asarray(Wq, np.float32).T)
    wkt = np.ascontiguousarray(np.asarray(Wk, np.float32).T)
    wpt = np.ascontiguousarray(np.asarray(Wp, np.float32).T)
    bqp = np.ascontiguousarray(np.asarray(bq, np.float32).reshape(FT, 128).T)
    bkp = np.ascontiguousarray(np.asarray(bk, np.float32).reshape(FT, 128).T)
    bc = np.zeros((33, 1024), np.float32)
    cols = np.arange(D)
    bc[cols // HD, cols] = 1.0                      # selector rows 0..11
    bc[32, :D] = np.asarray(bp, np.float32)         # output bias row (base 32)
    bc[32, D : D + 128] = 1.0                       # ones segment for bias matmul
    ident = np.eye(128, dtype=np.float32)
    base = {
        "wqt": wqt, "wkt": wkt, "wpt": wpt,
        "bqp": bqp, "bkp": bkp, "bc": bc, "ident": ident,
    }
    in_maps = []
    for c in range(N_CORES):
        xc = np.ascontiguousarray(x[BL * c : BL * (c + 1)].reshape(T, D).T)
        in_maps.append(dict(base, x=xc))
    return in_maps


def _run(in_maps, **kw):
    from concourse.bass_utils import run_bass_kernel_spmd

    nc = _get_nc()
    return run_bass_kernel_spmd(nc, in_maps, core_ids=list(range(N_CORES)), **kw)


def kernel(x, Wq, bq, Wk, bk, Wp, bp):
    in_maps = _prep_inputs(x, Wq, bq, Wk, bk, Wp, bp)
    res = _run(in_maps)
    out = np.concatenate(
        [r["out"].reshape(BL, N, D) for r in res.results], axis=0
    )
    return out.astype(np.float32)



# revision 10
# speedup vs baseline: 1.2237x; 1.2237x over previous
"""Trainium2 Bass kernel for nn_AttentionSeparateQKV (B=16, N=1024, D=768, H=12).

Data-parallel over batch: 8 NeuronCores x 2 batches each. Per core, per batch:
  Q/K projections (bf16 matmuls, feature-major PSUM, bias fused in the
  PSUM->SBUF copy on DVE).
  Scores computed TRANSPOSED: S^T[k, q] = (kT tile).T-matmul(qT), so the
  exp output (probs^T) is already in the AV-stationary orientation and no
  per-tile DMA transposes of probs are needed.
  exp on ACT with fused scale (1/sqrt(hd)) and range bias (-7.5*ln2); the
  bias cancels in normalization and keeps probs in fp8 range for the
  optional fp8 AV stage.
  V = K (reference reuses the K projection): v_ext = XBAR transpose of kT,
  laid out [128 keys, (kt, head) blocks, 80] with a ones column at 64 so
  the AV matmul also produces the softmax denominator for free.
  AV token-major: out[q, d] per head; normalization is a single DVE
  tensor_scalar divide by the denominator column (per-partition scalar).
  Out-projection from DMA-transposed out tiles; bias via DVE tensor_tensor.
"""

import sys

if "/opt/trn_rl_repo" not in sys.path:
    sys.path.insert(0, "/opt/trn_rl_repo")

import math

import numpy as np

B, N, D, H = 16, 1024, 768, 12
HD = D // H                # 64
N_CORES = 8
BL = B // N_CORES          # 2 batches per core
T = BL * N                 # 2048 tokens per core
FT = D // 128              # 6 feature tiles == head pairs
NKT = N // 128             # 8 key tiles per batch
NTT = N // 128             # 8 token tiles per batch
SCALE = float(HD) ** -0.5  # 0.125
CP = 7.5                   # probs = exp(SCALE*s - CP*ln2); cancels in softmax
EXP_BIAS = -CP * math.log(2.0)

PROBS_FP8 = False          # stage C switch: fp8e4 probs/v + DoubleRow AV

_NC_CACHE = []


def _build():
    import concourse.mybir as mybir
    import concourse.tile as tile
    from concourse import bacc

    F32 = mybir.dt.float32
    BF16 = mybir.dt.bfloat16
    FP8 = mybir.dt.float8e4
    EXP = mybir.ActivationFunctionType.Exp
    DIV = mybir.AluOpType.divide
    ADD = mybir.AluOpType.add
    DR = mybir.MatmulPerfMode.DoubleRow

    nc = bacc.Bacc("TRN2", target_bir_lowering=False, debug=False)

    x_d = nc.dram_tensor("x", [D, T], BF16, kind="ExternalInput").ap()
    wq_d = nc.dram_tensor("wqt", [D, D], BF16, kind="ExternalInput").ap()
    wk_d = nc.dram_tensor("wkt", [D, D], BF16, kind="ExternalInput").ap()
    wp_d = nc.dram_tensor("wpt", [D, D], BF16, kind="ExternalInput").ap()
    bq_d = nc.dram_tensor("bqp", [128, FT], F32, kind="ExternalInput").ap()
    bk_d = nc.dram_tensor("bkp", [128, FT], F32, kind="ExternalInput").ap()
    bp_d = nc.dram_tensor("bpp", [128, D], F32, kind="ExternalInput").ap()
    out_d = nc.dram_tensor("out", [T, D], F32, kind="ExternalOutput").ap()

    with tile.TileContext(nc) as tc:
        with (
            tc.tile_pool(name="const", bufs=1) as cpool,
            tc.tile_pool(name="xp", bufs=2) as xpool,
            tc.tile_pool(name="qk", bufs=2) as qkpool,
            tc.tile_pool(name="vx", bufs=2) as vpool,
            tc.tile_pool(name="probs", bufs=3) as prpool,
            tc.tile_pool(name="outsb", bufs=2) as opool,
            tc.tile_pool(name="outT", bufs=2) as otpool,
            tc.tile_pool(name="fin", bufs=3) as fpool,
            tc.tile_pool(name="rcp", bufs=4) as rpool,
            tc.tile_pool(name="ps_sc", bufs=2, space="PSUM") as ps_sc,
            tc.tile_pool(name="ps_sh", bufs=2, space="PSUM") as ps_sh,
        ):
            # ---- constants / weights ----
            bq_sb = cpool.tile([128, FT], F32, tag="bq")
            nc.sync.dma_start(bq_sb[:], bq_d[:])
            bk_sb = cpool.tile([128, FT], F32, tag="bk")
            nc.sync.dma_start(bk_sb[:], bk_d[:])
            bp_sb = cpool.tile([128, D], F32, tag="bp")
            nc.sync.dma_start(bp_sb[:], bp_d[:])
            ebias = cpool.tile([128, 1], F32, tag="ebias")
            nc.vector.memset(ebias[:], EXP_BIAS)

            wq_sb = cpool.tile([128, FT, D], BF16, tag="wq")
            wk_sb = cpool.tile([128, FT, D], BF16, tag="wk")
            wp_sb = cpool.tile([128, FT, D], BF16, tag="wp")

            def emit_xT(b):
                xT = xpool.tile([128, FT, N], BF16, tag="xT", name="xT")
                nc.gpsimd.dma_start(
                    xT[:],
                    x_d[:, b * N : (b + 1) * N].rearrange("(ko kp) t -> kp ko t", kp=128),
                )
                return xT

            def emit_proj_piece(xT, f, half, wsb, bias_sb, dst):
                """One projection psum tile: feature tile f, token half `half`."""
                ps = ps_sh.tile([128, 512], F32, tag="sh", name="ps")
                for ks in range(FT):
                    nc.tensor.matmul(
                        ps[:],
                        wsb[:, ks, 128 * f : 128 * (f + 1)],
                        xT[:, ks, 512 * half : 512 * (half + 1)],
                        start=(ks == 0),
                        stop=(ks == FT - 1),
                    )
                nc.vector.tensor_scalar_add(
                    dst[:, 512 * half : 512 * (half + 1)], ps[:], bias_sb[:, f : f + 1]
                )

            def emit_vext(kTf):
                """v_ext[p, e, kt, d] = kTf[64e+d, 128kt+p]; col 64 = ones.

                One XBAR transpose per head half: in [64, 1024] -> out
                [128 key, 8 kt, 64 d] written into the 80-col padded layout.
                """
                vext = vpool.tile([128, 2, NKT, 80], BF16, tag="vext", name="vext")
                nc.vector.memset(vext[:, :, :, 64:80], 0.0)
                nc.vector.memset(vext[:, :, :, 64:65], 1.0)
                for e in range(2):
                    nc.sync.dma_start_transpose(
                        vext[:, e, :, 0:64], kTf[64 * e : 64 * (e + 1), :]
                    )
                if PROBS_FP8:
                    v8 = vpool.tile([128, 2, NKT, 80], FP8, tag="v8", name="v8")
                    nc.vector.tensor_copy(v8[:], vext[:])
                    return v8
                return vext

            def emit_outproj(outT, b, tt):
                fin = fpool.tile([128, D], F32, tag="fin", name="fin")
                for nh in range(2):
                    ps = ps_sh.tile([128, 512], F32, tag="sh", name="ps")
                    for ks in range(FT):
                        nc.tensor.matmul(
                            ps[:, 0:384],
                            outT[:, ks, 128 * tt : 128 * (tt + 1)],
                            wp_sb[:, ks, 384 * nh : 384 * (nh + 1)],
                            start=(ks == 0),
                            stop=(ks == FT - 1),
                        )
                    nc.vector.tensor_tensor(
                        fin[:, 384 * nh : 384 * (nh + 1)],
                        ps[:, 0:384],
                        bp_sb[:, 384 * nh : 384 * (nh + 1)],
                        ADD,
                    )
                nc.sync.dma_start(
                    out_d[b * N + 128 * tt : b * N + 128 * (tt + 1), :], fin[:]
                )

            # Two filler queues of deferred PE work, consumed between
            # attention pipeline stages: proj_fill must drain within the
            # current f window (next scores need it emitted); bg_fill
            # (out-projection of the previous batch) can go anywhere.
            proj_fill = []
            bg_fill = []

            def pump(n):
                for _ in range(n):
                    if proj_fill:
                        proj_fill.pop(0)()
                    elif bg_fill:
                        bg_fill.pop(0)()

            def flush_av(pend):
                """AV + normalize for one (head, chunk) unit."""
                pr, vext, e, c, out_sb, h = pend
                for tt in range(4):
                    ttg = 4 * c + tt
                    av = ps_sh.tile([128, 512], F32, tag="sh", name="ps")
                    if PROBS_FP8:
                        for kp in range(NKT // 2):
                            nc.tensor.matmul(
                                av[:, 0:80],
                                pr[:, 2 * kp : 2 * kp + 2, 128 * tt : 128 * (tt + 1)],
                                vext[:, e, 2 * kp : 2 * kp + 2, :],
                                start=(kp == 0),
                                stop=(kp == NKT // 2 - 1),
                                perf_mode=DR,
                            )
                    else:
                        for kt in range(NKT):
                            nc.tensor.matmul(
                                av[:, 0:65],
                                pr[:, kt, 128 * tt : 128 * (tt + 1)],
                                vext[:, e, kt, 0:65],
                                start=(kt == 0),
                                stop=(kt == NKT - 1),
                            )
                    rcp = rpool.tile([128, 1], F32, tag="rcp", name="rcp")
                    nc.vector.reciprocal(rcp[:], av[:, 64:65])
                    nc.vector.tensor_scalar_mul(
                        out_sb[:, ttg, HD * h : HD * (h + 1)],
                        av[:, 0:64],
                        rcp[:, 0:1],
                    )

            # ---- prologue ----
            xT = emit_xT(0)
            nc.gpsimd.dma_start(wq_sb[:], wq_d.rearrange("(ko kp) m -> kp ko m", kp=128))
            nc.gpsimd.dma_start(wk_sb[:], wk_d.rearrange("(ko kp) m -> kp ko m", kp=128))
            qTf = qkpool.tile([128, N], BF16, tag="qTf", name="qTf")
            kTf = qkpool.tile([128, N], BF16, tag="kTf", name="kTf")
            for half in range(2):
                emit_proj_piece(xT, 0, half, wk_sb, bk_sb, kTf)
            vext = emit_vext(kTf)
            for half in range(2):
                emit_proj_piece(xT, 0, half, wq_sb, bq_sb, qTf)
            nc.gpsimd.dma_start(wp_sb[:], wp_d.rearrange("(ko kp) m -> kp ko m", kp=128))

            pend_av = None
            cur = (qTf, kTf, vext)
            xT_next = None

            for b in range(BL):
                if b > 0:
                    xT = xT_next
                out_sb = opool.tile([128, NTT, D], BF16, tag="out", name="out_sb")
                outT = otpool.tile([128, FT, N], BF16, tag="outT", name="outT")
                for f in range(FT):
                    qTf, kTf, vext = cur
                    # stage next projection (f+1, or next batch's f=0)
                    have_next = f < FT - 1 or b + 1 < BL
                    if f == 0 and b + 1 < BL:
                        xT_next = emit_xT(b + 1)
                    if have_next:
                        nf = (f + 1) % FT
                        nxt_x = xT if f < FT - 1 else xT_next
                        nxt_q = qkpool.tile([128, N], BF16, tag="qTf", name="qTf")
                        nxt_k = qkpool.tile([128, N], BF16, tag="kTf", name="kTf")
                        nxt_v = [None]
                        for half in range(2):
                            proj_fill.append(
                                lambda h2=half, k2=nxt_k, x2=nxt_x, f2=nf: emit_proj_piece(
                                    x2, f2, h2, wk_sb, bk_sb, k2
                                )
                            )
                        proj_fill.append(
                            lambda k2=nxt_k, v2=nxt_v: v2.__setitem__(0, emit_vext(k2))
                        )
                        for half in range(2):
                            proj_fill.append(
                                lambda h2=half, q2=nxt_q, x2=nxt_x, f2=nf: emit_proj_piece(
                                    x2, f2, h2, wq_sb, bq_sb, q2
                                )
                            )
                    for e in range(2):
                        h = 2 * f + e
                        for c in range(2):
                            pr = prpool.tile(
                                [128, NKT, 512], FP8 if PROBS_FP8 else BF16,
                                tag="pr", name="pr",
                            )
                            for k0, nk in ((0, 3), (3, 3), (6, 2)):
                                ps = ps_sc.tile([128, 3, 512], F32, tag="sc", name="ps")
                                for j in range(nk):
                                    kt = k0 + j
                                    nc.tensor.matmul(
                                        ps[:, j, :],
                                        kTf[64 * e : 64 * (e + 1), 128 * kt : 128 * (kt + 1)],
                                        qTf[64 * e : 64 * (e + 1), 512 * c : 512 * (c + 1)],
                                        start=True,
                                        stop=True,
                                    )
                                nc.scalar.activation(
                                    pr[:, k0 : k0 + nk, :],
                                    ps[:, 0:nk, :],
                                    EXP,
                                    scale=SCALE,
                                    bias=ebias[:],
                                )
                                pump(1)
                            if pend_av is not None:
                                flush_av(pend_av)
                            pend_av = (pr, vext, e, c, out_sb, h)
                            pump(1)
                    if have_next:
                        pump(len(proj_fill))  # force-complete next projection
                        cur = (nxt_q, nxt_k, nxt_v[0])
                # batch b attention fully emitted; defer its out-projection
                osb, oT, b2 = out_sb, outT, b
                for tt in range(NTT):
                    bg_fill.append(
                        lambda t2=tt, o2=osb, T2=oT: nc.sync.dma_start_transpose(
                            T2[:, :, 128 * t2 : 128 * (t2 + 1)], o2[:, t2, :]
                        )
                    )
                    bg_fill.append(
                        lambda t2=tt, T2=oT, b3=b2: emit_outproj(T2, b3, t2)
                    )
            flush_av(pend_av)
            while bg_fill:
                bg_fill.pop(0)()

    nc.compile()
    return nc


def _get_nc():
    if not _NC_CACHE:
        _NC_CACHE.append(_build())
    return _NC_CACHE[0]


def _to_np(a):
    try:
        return np.asarray(a)
    except Exception:
        import jax

        return np.asarray(jax.device_get(a))


def _prep_inputs(x, Wq, bq, Wk, bk, Wp, bp):
    import ml_dtypes

    bf16 = ml_dtypes.bfloat16
    x, Wq, bq, Wk, bk, Wp, bp = (_to_np(a) for a in (x, Wq, bq, Wk, bk, Wp, bp))
    wqt = np.ascontiguousarray(np.asarray(Wq, np.float32).T.astype(bf16))
    wkt = np.ascontiguousarray(np.asarray(Wk, np.float32).T.astype(bf16))
    wpt = np.ascontiguousarray(np.asarray(Wp, np.float32).T.astype(bf16))
    bqp = np.ascontiguousarray(np.asarray(bq, np.float32).reshape(FT, 128).T)
    bkp = np.ascontiguousarray(np.asarray(bk, np.float32).reshape(FT, 128).T)
    bpp = np.ascontiguousarray(
        np.broadcast_to(np.asarray(bp, np.float32)[None, :], (128, D))
    )
    base = {
        "wqt": wqt, "wkt": wkt, "wpt": wpt,
        "bqp": bqp, "bkp": bkp, "bpp": bpp,
    }
    in_maps = []
    xf = np.asarray(x, np.float32)
    for c in range(N_CORES):
        xc = np.ascontiguousarray(
            xf[BL * c : BL * (c + 1)].reshape(T, D).T.astype(bf16)
        )
        in_maps.append(dict(base, x=xc))
    return in_maps


def _run(in_maps, **kw):
    from concourse.bass_utils import run_bass_kernel_spmd

    nc = _get_nc()
    return run_bass_kernel_spmd(nc, in_maps, core_ids=list(range(N_CORES)), **kw)


def kernel(x, Wq, bq, Wk, bk, Wp, bp):
    in_maps = _prep_inputs(x, Wq, bq, Wk, bk, Wp, bp)
    res = _run(in_maps)
    out = np.concatenate(
        [r["out"].reshape(BL, N, D) for r in res.results], axis=0
    )
    return out.astype(np.float32)


# revision 11
# speedup vs baseline: 1.2525x; 1.0235x over previous
"""Trainium2 Bass kernel for nn_AttentionSeparateQKV (B=16, N=1024, D=768, H=12).

Data-parallel over batch: 8 NeuronCores x 2 batches each. Per core, per batch:
  Q/K projections (bf16 matmuls, feature-major PSUM, bias fused in the
  PSUM->SBUF copy on DVE).
  Scores computed TRANSPOSED: S^T[k, q] = (kT tile).T-matmul(qT), so the
  exp output (probs^T) is already in the AV-stationary orientation and no
  per-tile DMA transposes of probs are needed.
  exp on ACT with fused scale (1/sqrt(hd)) and range bias (-7.5*ln2); the
  bias cancels in normalization and keeps probs in fp8 range for the
  optional fp8 AV stage.
  V = K (reference reuses the K projection): v_ext = XBAR transpose of kT,
  laid out [128 keys, (kt, head) blocks, 80] with a ones column at 64 so
  the AV matmul also produces the softmax denominator for free.
  AV token-major: out[q, d] per head; normalization is a single DVE
  tensor_scalar divide by the denominator column (per-partition scalar).
  Out-projection from DMA-transposed out tiles; bias via DVE tensor_tensor.
"""

import sys

if "/opt/trn_rl_repo" not in sys.path:
    sys.path.insert(0, "/opt/trn_rl_repo")

import math

import numpy as np

B, N, D, H = 16, 1024, 768, 12
HD = D // H                # 64
N_CORES = 8
BL = B // N_CORES          # 2 batches per core
T = BL * N                 # 2048 tokens per core
FT = D // 128              # 6 feature tiles == head pairs
NKT = N // 128             # 8 key tiles per batch
NTT = N // 128             # 8 token tiles per batch
SCALE = float(HD) ** -0.5  # 0.125
CP = 7.5                   # probs = exp(SCALE*s - CP*ln2); cancels in softmax
EXP_BIAS = -CP * math.log(2.0)

PROBS_FP8 = True          # stage C switch: fp8e4 probs/v + DoubleRow AV

_NC_CACHE = []


def _build():
    import concourse.mybir as mybir
    import concourse.tile as tile
    from concourse import bacc

    F32 = mybir.dt.float32
    BF16 = mybir.dt.bfloat16
    FP8 = mybir.dt.float8e4
    EXP = mybir.ActivationFunctionType.Exp
    DIV = mybir.AluOpType.divide
    ADD = mybir.AluOpType.add
    DR = mybir.MatmulPerfMode.DoubleRow

    nc = bacc.Bacc("TRN2", target_bir_lowering=False, debug=False)

    x_d = nc.dram_tensor("x", [D, T], BF16, kind="ExternalInput").ap()
    wq_d = nc.dram_tensor("wqt", [D, D], BF16, kind="ExternalInput").ap()
    wk_d = nc.dram_tensor("wkt", [D, D], BF16, kind="ExternalInput").ap()
    wp_d = nc.dram_tensor("wpt", [D, D], BF16, kind="ExternalInput").ap()
    bq_d = nc.dram_tensor("bqp", [128, FT], F32, kind="ExternalInput").ap()
    bk_d = nc.dram_tensor("bkp", [128, FT], F32, kind="ExternalInput").ap()
    bp_d = nc.dram_tensor("bpp", [128, D], F32, kind="ExternalInput").ap()
    out_d = nc.dram_tensor("out", [T, D], F32, kind="ExternalOutput").ap()

    with tile.TileContext(nc) as tc:
        with (
            tc.tile_pool(name="const", bufs=1) as cpool,
            tc.tile_pool(name="xp", bufs=2) as xpool,
            tc.tile_pool(name="qk", bufs=2) as qkpool,
            tc.tile_pool(name="vx", bufs=2) as vpool,
            tc.tile_pool(name="probs", bufs=3) as prpool,
            tc.tile_pool(name="outsb", bufs=2) as opool,
            tc.tile_pool(name="outT", bufs=2) as otpool,
            tc.tile_pool(name="fin", bufs=3) as fpool,
            tc.tile_pool(name="rcp", bufs=4) as rpool,
            tc.tile_pool(name="ps_sc", bufs=2, space="PSUM") as ps_sc,
            tc.tile_pool(name="ps_sh", bufs=2, space="PSUM") as ps_sh,
        ):
            # ---- constants / weights ----
            bq_sb = cpool.tile([128, FT], F32, tag="bq")
            nc.sync.dma_start(bq_sb[:], bq_d[:])
            bk_sb = cpool.tile([128, FT], F32, tag="bk")
            nc.sync.dma_start(bk_sb[:], bk_d[:])
            bp_sb = cpool.tile([128, D], F32, tag="bp")
            nc.sync.dma_start(bp_sb[:], bp_d[:])
            ebias = cpool.tile([128, 1], F32, tag="ebias")
            nc.vector.memset(ebias[:], EXP_BIAS)

            wq_sb = cpool.tile([128, FT, D], BF16, tag="wq")
            wk_sb = cpool.tile([128, FT, D], BF16, tag="wk")
            wp_sb = cpool.tile([128, FT, D], BF16, tag="wp")

            def emit_xT(b):
                xT = xpool.tile([128, FT, N], BF16, tag="xT", name="xT")
                nc.gpsimd.dma_start(
                    xT[:],
                    x_d[:, b * N : (b + 1) * N].rearrange("(ko kp) t -> kp ko t", kp=128),
                )
                return xT

            def emit_proj_piece(xT, f, half, wsb, bias_sb, dst):
                """One projection psum tile: feature tile f, token half `half`."""
                ps = ps_sh.tile([128, 512], F32, tag="sh", name="ps")
                for ks in range(FT):
                    nc.tensor.matmul(
                        ps[:],
                        wsb[:, ks, 128 * f : 128 * (f + 1)],
                        xT[:, ks, 512 * half : 512 * (half + 1)],
                        start=(ks == 0),
                        stop=(ks == FT - 1),
                    )
                nc.vector.tensor_scalar_add(
                    dst[:, 512 * half : 512 * (half + 1)], ps[:], bias_sb[:, f : f + 1]
                )

            def emit_vext(kTf):
                """v_ext[p, e, kt, d] = kTf[64e+d, 128kt+p]; col 64 = ones.

                One XBAR transpose per head half: in [64, 1024] -> out
                [128 key, 8 kt, 64 d] written into the 80-col padded layout.
                """
                vext = vpool.tile([128, 2, NKT, 80], BF16, tag="vext", name="vext")
                nc.vector.memset(vext[:, :, :, 64:80], 0.0)
                nc.vector.memset(vext[:, :, :, 64:65], 1.0)
                for e in range(2):
                    nc.sync.dma_start_transpose(
                        vext[:, e, :, 0:64], kTf[64 * e : 64 * (e + 1), :]
                    )
                if PROBS_FP8:
                    v8 = vpool.tile([128, 2, NKT, 80], FP8, tag="v8", name="v8")
                    nc.vector.tensor_copy(v8[:], vext[:])
                    return v8
                return vext

            def emit_outproj(outT, b, tt):
                fin = fpool.tile([128, D], F32, tag="fin", name="fin")
                for nh in range(2):
                    ps = ps_sh.tile([128, 512], F32, tag="sh", name="ps")
                    for ks in range(FT):
                        nc.tensor.matmul(
                            ps[:, 0:384],
                            outT[:, ks, 128 * tt : 128 * (tt + 1)],
                            wp_sb[:, ks, 384 * nh : 384 * (nh + 1)],
                            start=(ks == 0),
                            stop=(ks == FT - 1),
                        )
                    nc.vector.tensor_tensor(
                        fin[:, 384 * nh : 384 * (nh + 1)],
                        ps[:, 0:384],
                        bp_sb[:, 384 * nh : 384 * (nh + 1)],
                        ADD,
                    )
                nc.sync.dma_start(
                    out_d[b * N + 128 * tt : b * N + 128 * (tt + 1), :], fin[:]
                )

            # Two filler queues of deferred PE work, consumed between
            # attention pipeline stages: proj_fill must drain within the
            # current f window (next scores need it emitted); bg_fill
            # (out-projection of the previous batch) can go anywhere.
            proj_fill = []
            bg_fill = []

            def pump(n):
                for _ in range(n):
                    if proj_fill:
                        proj_fill.pop(0)()
                    elif bg_fill:
                        bg_fill.pop(0)()

            def flush_av(pend):
                """AV + normalize for one (head, chunk) unit."""
                pr, vext, e, c, out_sb, h = pend
                for tt in range(4):
                    ttg = 4 * c + tt
                    av = ps_sh.tile([128, 512], F32, tag="sh", name="ps")
                    if PROBS_FP8:
                        for kp in range(NKT // 2):
                            nc.tensor.matmul(
                                av[:, 0:80],
                                pr[:, 2 * kp : 2 * kp + 2, 128 * tt : 128 * (tt + 1)],
                                vext[:, e, 2 * kp : 2 * kp + 2, :],
                                start=(kp == 0),
                                stop=(kp == NKT // 2 - 1),
                                perf_mode=DR,
                            )
                    else:
                        for kt in range(NKT):
                            nc.tensor.matmul(
                                av[:, 0:65],
                                pr[:, kt, 128 * tt : 128 * (tt + 1)],
                                vext[:, e, kt, 0:65],
                                start=(kt == 0),
                                stop=(kt == NKT - 1),
                            )
                    rcp = rpool.tile([128, 1], F32, tag="rcp", name="rcp")
                    nc.vector.reciprocal(rcp[:], av[:, 64:65])
                    nc.vector.tensor_scalar_mul(
                        out_sb[:, ttg, HD * h : HD * (h + 1)],
                        av[:, 0:64],
                        rcp[:, 0:1],
                    )

            # ---- prologue ----
            xT = emit_xT(0)
            nc.gpsimd.dma_start(wq_sb[:], wq_d.rearrange("(ko kp) m -> kp ko m", kp=128))
            nc.gpsimd.dma_start(wk_sb[:], wk_d.rearrange("(ko kp) m -> kp ko m", kp=128))
            qTf = qkpool.tile([128, N], BF16, tag="qTf", name="qTf")
            kTf = qkpool.tile([128, N], BF16, tag="kTf", name="kTf")
            for half in range(2):
                emit_proj_piece(xT, 0, half, wk_sb, bk_sb, kTf)
            vext = emit_vext(kTf)
            for half in range(2):
                emit_proj_piece(xT, 0, half, wq_sb, bq_sb, qTf)
            nc.gpsimd.dma_start(wp_sb[:], wp_d.rearrange("(ko kp) m -> kp ko m", kp=128))

            pend_av = None
            cur = (qTf, kTf, vext)
            xT_next = None

            for b in range(BL):
                if b > 0:
                    xT = xT_next
                out_sb = opool.tile([128, NTT, D], BF16, tag="out", name="out_sb")
                outT = otpool.tile([128, FT, N], BF16, tag="outT", name="outT")
                for f in range(FT):
                    qTf, kTf, vext = cur
                    # stage next projection (f+1, or next batch's f=0)
                    have_next = f < FT - 1 or b + 1 < BL
                    if f == 0 and b + 1 < BL:
                        xT_next = emit_xT(b + 1)
                    if have_next:
                        nf = (f + 1) % FT
                        nxt_x = xT if f < FT - 1 else xT_next
                        nxt_q = qkpool.tile([128, N], BF16, tag="qTf", name="qTf")
                        nxt_k = qkpool.tile([128, N], BF16, tag="kTf", name="kTf")
                        nxt_v = [None]
                        for half in range(2):
                            proj_fill.append(
                                lambda h2=half, k2=nxt_k, x2=nxt_x, f2=nf: emit_proj_piece(
                                    x2, f2, h2, wk_sb, bk_sb, k2
                                )
                            )
                        proj_fill.append(
                            lambda k2=nxt_k, v2=nxt_v: v2.__setitem__(0, emit_vext(k2))
                        )
                        for half in range(2):
                            proj_fill.append(
                                lambda h2=half, q2=nxt_q, x2=nxt_x, f2=nf: emit_proj_piece(
                                    x2, f2, h2, wq_sb, bq_sb, q2
                                )
                            )
                    for e in range(2):
                        h = 2 * f + e
                        for c in range(2):
                            pr = prpool.tile(
                                [128, NKT, 512], FP8 if PROBS_FP8 else BF16,
                                tag="pr", name="pr",
                            )
                            for k0, nk in ((0, 3), (3, 3), (6, 2)):
                                ps = ps_sc.tile([128, 3, 512], F32, tag="sc", name="ps")
                                for j in range(nk):
                                    kt = k0 + j
                                    nc.tensor.matmul(
                                        ps[:, j, :],
                                        kTf[64 * e : 64 * (e + 1), 128 * kt : 128 * (kt + 1)],
                                        qTf[64 * e : 64 * (e + 1), 512 * c : 512 * (c + 1)],
                                        start=True,
                                        stop=True,
                                    )
                                nc.scalar.activation(
                                    pr[:, k0 : k0 + nk, :],
                                    ps[:, 0:nk, :],
                                    EXP,
                                    scale=SCALE,
                                    bias=ebias[:],
                                )
                                pump(1)
                            if pend_av is not None:
                                flush_av(pend_av)
                            pend_av = (pr, vext, e, c, out_sb, h)
                            pump(1)
                    if have_next:
                        pump(len(proj_fill))  # force-complete next projection
                        cur = (nxt_q, nxt_k, nxt_v[0])
                # batch b attention fully emitted; defer its out-projection
                osb, oT, b2 = out_sb, outT, b
                for tt in range(NTT):
                    bg_fill.append(
                        lambda t2=tt, o2=osb, T2=oT: nc.sync.dma_start_transpose(
                            T2[:, :, 128 * t2 : 128 * (t2 + 1)], o2[:, t2, :]
                        )
                    )
                    bg_fill.append(
                        lambda t2=tt, T2=oT, b3=b2: emit_outproj(T2, b3, t2)
                    )
            flush_av(pend_av)
            while bg_fill:
                bg_fill.pop(0)()

    nc.compile()
    return nc


def _get_nc():
    if not _NC_CACHE:
        _NC_CACHE.append(_build())
    return _NC_CACHE[0]


def _to_np(a):
    try:
        return np.asarray(a)
    except Exception:
        import jax

        return np.asarray(jax.device_get(a))


def _prep_inputs(x, Wq, bq, Wk, bk, Wp, bp):
    import ml_dtypes

    bf16 = ml_dtypes.bfloat16
    x, Wq, bq, Wk, bk, Wp, bp = (_to_np(a) for a in (x, Wq, bq, Wk, bk, Wp, bp))
    wqt = np.ascontiguousarray(np.asarray(Wq, np.float32).T.astype(bf16))
    wkt = np.ascontiguousarray(np.asarray(Wk, np.float32).T.astype(bf16))
    wpt = np.ascontiguousarray(np.asarray(Wp, np.float32).T.astype(bf16))
    bqp = np.ascontiguousarray(np.asarray(bq, np.float32).reshape(FT, 128).T)
    bkp = np.ascontiguousarray(np.asarray(bk, np.float32).reshape(FT, 128).T)
    bpp = np.ascontiguousarray(
        np.broadcast_to(np.asarray(bp, np.float32)[None, :], (128, D))
    )
    base = {
        "wqt": wqt, "wkt": wkt, "wpt": wpt,
        "bqp": bqp, "bkp": bkp, "bpp": bpp,
    }
    in_maps = []
    xf = np.asarray(x, np.float32)
    for c in range(N_CORES):
        xc = np.ascontiguousarray(
            xf[BL * c : BL * (c + 1)].reshape(T, D).T.astype(bf16)
        )
        in_maps.append(dict(base, x=xc))
    return in_maps


def _run(in_maps, **kw):
    from concourse.bass_utils import run_bass_kernel_spmd

    nc = _get_nc()
    return run_bass_kernel_spmd(nc, in_maps, core_ids=list(range(N_CORES)), **kw)


def kernel(x, Wq, bq, Wk, bk, Wp, bp):
    in_maps = _prep_inputs(x, Wq, bq, Wk, bk, Wp, bp)
    res = _run(in_maps)
    out = np.concatenate(
        [r["out"].reshape(BL, N, D) for r in res.results], axis=0
    )
    return out.astype(np.float32)


# revision 36
# speedup vs baseline: 1.3662x; 1.0908x over previous
"""Trainium2 Bass kernel for nn_AttentionSeparateQKV (B=16, N=1024, D=768, H=12).

Data-parallel over batch: 8 NeuronCores x 2 batches each. Per core, per batch:
  Q/K projections (bf16 matmuls, feature-major PSUM, bias fused in the
  PSUM->SBUF copy on DVE).
  Scores computed TRANSPOSED: S^T[k, q] = (kT tile).T-matmul(qT), so the
  exp output (probs^T) is already in the AV-stationary orientation and no
  per-tile DMA transposes of probs are needed.
  exp on ACT with fused scale (1/sqrt(hd)) and range bias (-7.5*ln2); the
  bias cancels in normalization and keeps probs in fp8 range for the
  optional fp8 AV stage.
  V = K (reference reuses the K projection): v_ext = XBAR transpose of kT,
  laid out [128 keys, (kt, head) blocks, 80] with a ones column at 64 so
  the AV matmul also produces the softmax denominator for free.
  AV token-major: out[q, d] per head; normalization is a single DVE
  tensor_scalar divide by the denominator column (per-partition scalar).
  Out-projection from DMA-transposed out tiles; bias via DVE tensor_tensor.
"""

import sys

if "/opt/trn_rl_repo" not in sys.path:
    sys.path.insert(0, "/opt/trn_rl_repo")

import math

import numpy as np

B, N, D, H = 16, 1024, 768, 12
HD = D // H                # 64
N_CORES = 8
BL = B // N_CORES          # 2 batches per core
T = BL * N                 # 2048 tokens per core
FT = D // 128              # 6 feature tiles == head pairs
NKT = N // 128             # 8 key tiles per batch
NTT = N // 128             # 8 token tiles per batch
SCALE = float(HD) ** -0.5  # 0.125
CP = 7.5                   # probs = exp(SCALE*s - CP*ln2); cancels in softmax
EXP_BIAS = -CP * math.log(2.0)

PROBS_FP8 = True          # stage C switch: fp8e4 probs/v + DoubleRow AV

_NC_CACHE = []


def _build():
    import concourse.mybir as mybir
    import concourse.tile as tile
    from concourse import bacc

    F32 = mybir.dt.float32
    BF16 = mybir.dt.bfloat16
    FP8 = mybir.dt.float8e4
    EXP = mybir.ActivationFunctionType.Exp
    DIV = mybir.AluOpType.divide
    ADD = mybir.AluOpType.add
    DR = mybir.MatmulPerfMode.DoubleRow

    nc = bacc.Bacc("TRN2", target_bir_lowering=False, debug=False)

    x_d = nc.dram_tensor("x", [D, T], BF16, kind="ExternalInput").ap()
    wq_d = nc.dram_tensor("wqt", [D, D], BF16, kind="ExternalInput").ap()
    wk_d = nc.dram_tensor("wkt", [D, D], BF16, kind="ExternalInput").ap()
    wp_d = nc.dram_tensor("wpt", [D, D], BF16, kind="ExternalInput").ap()
    bq_d = nc.dram_tensor("bqp", [128, FT], F32, kind="ExternalInput").ap()
    bk_d = nc.dram_tensor("bkp", [128, FT], F32, kind="ExternalInput").ap()
    bp_d = nc.dram_tensor("bpp", [128, D], F32, kind="ExternalInput").ap()
    out_d = nc.dram_tensor("out", [T, D], F32, kind="ExternalOutput").ap()

    with tile.TileContext(nc) as tc:
        with (
            tc.tile_pool(name="const", bufs=1) as cpool,
            tc.tile_pool(name="xp", bufs=2) as xpool,
            tc.tile_pool(name="qk", bufs=2) as qkpool,
            tc.tile_pool(name="vx", bufs=2) as vpool,
            tc.tile_pool(name="probs", bufs=3) as prpool,
            tc.tile_pool(name="outsb", bufs=2) as opool,
            tc.tile_pool(name="outT", bufs=2) as otpool,
            tc.tile_pool(name="fin", bufs=3) as fpool,
            tc.tile_pool(name="rcp", bufs=4) as rpool,
            tc.tile_pool(name="ps_sc", bufs=2, space="PSUM") as ps_sc,
            tc.tile_pool(name="ps_sh", bufs=2, space="PSUM") as ps_sh,
        ):
            # ---- constants (small loads on gpsimd; bulk loads come first on
            # the shared DMA device via sync/scalar queues) ----
            bq_sb = cpool.tile([128, FT], F32, tag="bq")
            nc.gpsimd.dma_start(bq_sb[:], bq_d[:])
            bk_sb = cpool.tile([128, FT], F32, tag="bk")
            nc.gpsimd.dma_start(bk_sb[:], bk_d[:])
            bp_sb = cpool.tile([128, D], F32, tag="bp")
            nc.gpsimd.dma_start(bp_sb[:], bp_d[:])
            ebias = cpool.tile([128, 1], F32, tag="ebias")
            nc.vector.memset(ebias[:], EXP_BIAS)

            wq_sb = cpool.tile([128, FT, D], BF16, tag="wq")
            wk_sb = cpool.tile([128, FT, D], BF16, tag="wk")
            wp_sb = cpool.tile([128, FT, D], BF16, tag="wp")

            def emit_xT(b):
                xT = xpool.tile([128, FT, N], BF16, tag="xT", name="xT")
                nc.gpsimd.dma_start(
                    xT[:],
                    x_d[:, b * N : (b + 1) * N].rearrange("(ko kp) t -> kp ko t", kp=128),
                )
                return xT

            def emit_proj_piece(xT, f, half, wsb, bias_sb, dst):
                """One projection psum tile: feature tile f, token half `half`."""
                ps = ps_sh.tile([128, 512], F32, tag="sh", name="ps")
                for ks in range(FT):
                    nc.tensor.matmul(
                        ps[:],
                        wsb[:, ks, 128 * f : 128 * (f + 1)],
                        xT[:, ks, 512 * half : 512 * (half + 1)],
                        start=(ks == 0),
                        stop=(ks == FT - 1),
                    )
                nc.vector.tensor_scalar_add(
                    dst[:, 512 * half : 512 * (half + 1)], ps[:], bias_sb[:, f : f + 1]
                )

            def emit_vext(kTf):
                """v_ext[p, e, kt, d] = kTf[64e+d, 128kt+p]; col 64 = ones.

                One XBAR transpose per head half: in [64, 1024] -> out
                [128 key, 8 kt, 64 d] written into the 80-col padded layout.
                """
                vext = vpool.tile([128, 2, NKT, 80], BF16, tag="vext", name="vext")
                nc.gpsimd.memset(vext[:, :, :, 64:80], 0.0)
                nc.gpsimd.memset(vext[:, :, :, 64:65], 1.0)
                for e in range(2):
                    nc.sync.dma_start_transpose(
                        vext[:, e, :, 0:64], kTf[64 * e : 64 * (e + 1), :]
                    )
                if PROBS_FP8:
                    v8 = vpool.tile([128, 2, NKT, 80], FP8, tag="v8", name="v8")
                    nc.gpsimd.dma_start(v8[:], vext[:])  # casting DMA (SWDGE)
                    return v8
                return vext

            def emit_outproj(outT, b, tt):
                fin = fpool.tile([128, D], F32, tag="fin", name="fin")
                for nh in range(2):
                    ps = ps_sh.tile([128, 512], F32, tag="sh", name="ps")
                    for ks in range(FT):
                        nc.tensor.matmul(
                            ps[:, 0:384],
                            outT[:, ks, 128 * tt : 128 * (tt + 1)],
                            wp_sb[:, ks, 384 * nh : 384 * (nh + 1)],
                            start=(ks == 0),
                            stop=(ks == FT - 1),
                        )
                    nc.vector.tensor_tensor(
                        fin[:, 384 * nh : 384 * (nh + 1)],
                        ps[:, 0:384],
                        bp_sb[:, 384 * nh : 384 * (nh + 1)],
                        ADD,
                    )
                nc.sync.dma_start(
                    out_d[b * N + 128 * tt : b * N + 128 * (tt + 1), :], fin[:]
                )

            # Filler queues of deferred PE work as (cost_cycles, fn) pairs,
            # consumed between exp emissions in sub-0.7us pieces so PE always
            # returns to scores production within ACT's consumption cadence.
            proj_fill = []
            bg_fill = []

            def pump(budget=1700):
                while budget > 0 and (proj_fill or bg_fill):
                    cost, fn = (proj_fill or bg_fill).pop(0)
                    fn()
                    budget -= cost

            def drain_proj():
                while proj_fill:
                    proj_fill.pop(0)[1]()

            def stage_proj(xT2, f2, half, wsb, bias_sb, dst):
                box = {}

                def sub_a():
                    ps = ps_sh.tile([128, 512], F32, tag="sh", name="ps")
                    box["ps"] = ps
                    for ks in range(3):
                        nc.tensor.matmul(
                            ps[:],
                            wsb[:, ks, 128 * f2 : 128 * (f2 + 1)],
                            xT2[:, ks, 512 * half : 512 * (half + 1)],
                            start=(ks == 0),
                            stop=False,
                        )

                def sub_b():
                    ps = box["ps"]
                    for ks in range(3, FT):
                        nc.tensor.matmul(
                            ps[:],
                            wsb[:, ks, 128 * f2 : 128 * (f2 + 1)],
                            xT2[:, ks, 512 * half : 512 * (half + 1)],
                            start=False,
                            stop=(ks == FT - 1),
                        )
                    nc.vector.tensor_scalar_add(
                        dst[:, 512 * half : 512 * (half + 1)], ps[:],
                        bias_sb[:, f2 : f2 + 1],
                    )

                proj_fill.append((1536, sub_a))
                proj_fill.append((1536, sub_b))

            def stage_outproj(out_sb, outT, b2, tt):
                nc.sync.dma_start_transpose(
                    outT[:, :, 128 * tt : 128 * (tt + 1)], out_sb[:, tt, :]
                )
                fbox = {}

                def fin_alloc():
                    fbox["fin"] = fpool.tile([128, D], F32, tag="fin", name="fin")

                def mk_sub(nh, part):
                    def sub():
                        if nh == 0 and part == 0:
                            fin_alloc()
                        if part == 0:
                            ps = ps_sh.tile([128, 512], F32, tag="sh", name="ps")
                            fbox["ps"] = ps
                        ps = fbox["ps"]
                        for ks in range(3 * part, 3 * part + 3):
                            nc.tensor.matmul(
                                ps[:, 0:384],
                                outT[:, ks, 128 * tt : 128 * (tt + 1)],
                                wp_sb[:, ks, 384 * nh : 384 * (nh + 1)],
                                start=(ks == 0),
                                stop=(ks == FT - 1),
                            )
                        if part == 1:
                            nc.vector.tensor_tensor(
                                fbox["fin"][:, 384 * nh : 384 * (nh + 1)],
                                ps[:, 0:384],
                                bp_sb[:, 384 * nh : 384 * (nh + 1)],
                                ADD,
                            )
                            if nh == 1:
                                nc.sync.dma_start(
                                    out_d[b2 * N + 128 * tt : b2 * N + 128 * (tt + 1), :],
                                    fbox["fin"][:],
                                )
                    return sub

                for nh in range(2):
                    for part in range(2):
                        bg_fill.append((1152, mk_sub(nh, part)))

            def flush_av(pend):
                """AV + normalize for one (head, chunk) unit."""
                pr, vext, e, c, out_sb, outT, b2, h = pend
                for tt in range(4):
                    ttg = 4 * c + tt
                    av = ps_sh.tile([128, 512], F32, tag="sh", name="ps")
                    if PROBS_FP8:
                        for kp in range(NKT // 2):
                            nc.tensor.matmul(
                                av[:, 0:80],
                                pr[:, 2 * kp : 2 * kp + 2, 128 * tt : 128 * (tt + 1)],
                                vext[:, e, 2 * kp : 2 * kp + 2, :],
                                start=(kp == 0),
                                stop=(kp == NKT // 2 - 1),
                                perf_mode=DR,
                            )
                    else:
                        for kt in range(NKT):
                            nc.tensor.matmul(
                                av[:, 0:65],
                                pr[:, kt, 128 * tt : 128 * (tt + 1)],
                                vext[:, e, kt, 0:65],
                                start=(kt == 0),
                                stop=(kt == NKT - 1),
                            )
                    rcp = rpool.tile([128, 1], F32, tag="rcp", name="rcp")
                    nc.vector.reciprocal(rcp[:], av[:, 64:65])
                    nc.vector.tensor_scalar_mul(
                        out_sb[:, ttg, HD * h : HD * (h + 1)],
                        av[:, 0:64],
                        rcp[:, 0:1],
                    )
                if h == H - 1:
                    # chunk c of batch b2 is complete: queue its out-projection
                    for tt in range(4 * c, 4 * c + 4):
                        stage_outproj(out_sb, outT, b2, tt)

            # ---- prologue (xT + wk first so K projection starts earliest) ----
            xT = xpool.tile([128, FT, N], BF16, tag="xT", name="xT")
            nc.sync.dma_start(
                xT[:], x_d[:, 0:N].rearrange("(ko kp) t -> kp ko t", kp=128)
            )
            wk_r = wk_d.rearrange("(ko kp) m -> kp ko m", kp=128)
            wq_r = wq_d.rearrange("(ko kp) m -> kp ko m", kp=128)
            nc.scalar.dma_start(wk_sb[:, :, 0:384], wk_r[:, :, 0:384])
            nc.scalar.dma_start(wq_sb[:, :, 0:384], wq_r[:, :, 0:384])
            nc.scalar.dma_start(wk_sb[:, :, 384:768], wk_r[:, :, 384:768])
            nc.scalar.dma_start(wq_sb[:, :, 384:768], wq_r[:, :, 384:768])
            # PE p-state warmup while the loads are in flight
            warm = cpool.tile([128, 512], BF16, tag="warm")
            nc.vector.memset(warm[:], 0.0)
            for i in range(22):
                ps = ps_sc.tile([128, 3, 512], F32, tag="sc", name="ps")
                nc.tensor.matmul(
                    ps[:, i % 3, :], warm[:, 0:128], warm[:], start=True, stop=True
                )
            qTf = qkpool.tile([128, N], BF16, tag="qTf", name="qTf")
            kTf = qkpool.tile([128, N], BF16, tag="kTf", name="kTf")
            for half in range(2):
                emit_proj_piece(xT, 0, half, wk_sb, bk_sb, kTf)
            emit_proj_piece(xT, 0, 0, wq_sb, bq_sb, qTf)
            vext = emit_vext(kTf)
            emit_proj_piece(xT, 0, 1, wq_sb, bq_sb, qTf)
            nc.scalar.dma_start(wp_sb[:], wp_d.rearrange("(ko kp) m -> kp ko m", kp=128))

            pend_av = []
            cur = (qTf, kTf, vext)
            xT_next = None

            for b in range(BL):
                if b > 0:
                    xT = xT_next
                out_sb = opool.tile([128, NTT, D], BF16, tag="out", name="out_sb")
                outT = otpool.tile([128, FT, N], BF16, tag="outT", name="outT")
                for f in range(FT):
                    qTf, kTf, vext = cur
                    # stage next projection (f+1, or next batch's f=0)
                    have_next = f < FT - 1 or b + 1 < BL
                    if f == 2 and b + 1 < BL:
                        xT_next = emit_xT(b + 1)
                    if have_next:
                        nf = (f + 1) % FT
                        nxt_x = xT if f < FT - 1 else xT_next
                        nxt_q = qkpool.tile([128, N], BF16, tag="qTf", name="qTf")
                        nxt_k = qkpool.tile([128, N], BF16, tag="kTf", name="kTf")
                        nxt_v = [None]
                        stage_proj(nxt_x, nf, 0, wk_sb, bk_sb, nxt_k)
                        stage_proj(nxt_x, nf, 1, wk_sb, bk_sb, nxt_k)
                        stage_proj(nxt_x, nf, 0, wq_sb, bq_sb, nxt_q)
                        proj_fill.append(
                            (100, lambda k2=nxt_k, v2=nxt_v: v2.__setitem__(0, emit_vext(k2)))
                        )
                        stage_proj(nxt_x, nf, 1, wq_sb, bq_sb, nxt_q)
                    for c in range(2):
                        for e in range(2):
                            h = 2 * f + e
                            pr = prpool.tile(
                                [128, NKT, 512], FP8 if PROBS_FP8 else BF16,
                                tag="pr", name="pr",
                            )
                            for k0, nk in ((0, 3), (3, 3), (6, 2)):
                                ps = ps_sc.tile([128, 3, 512], F32, tag="sc", name="ps")
                                for j in range(nk):
                                    kt = k0 + j
                                    nc.tensor.matmul(
                                        ps[:, j, :],
                                        kTf[64 * e : 64 * (e + 1), 128 * kt : 128 * (kt + 1)],
                                        qTf[64 * e : 64 * (e + 1), 512 * c : 512 * (c + 1)],
                                        start=True,
                                        stop=True,
                                    )
                                nc.scalar.activation(
                                    pr[:, k0 : k0 + nk, :],
                                    ps[:, 0:nk, :],
                                    EXP,
                                    scale=SCALE,
                                    bias=ebias[:],
                                )
                                budget = 2300 if have_next else 4600
                                if k0 == 0:
                                    pump(budget)
                                elif k0 == 3:
                                    pump(budget)
                                    if len(pend_av) >= 2:
                                        flush_av(pend_av.pop(0))
                            pend_av.append((pr, vext, e, c, out_sb, outT, b, h))
                    if have_next:
                        drain_proj()
                        cur = (nxt_q, nxt_k, nxt_v[0])
            for p_ in pend_av:
                flush_av(p_)
            while bg_fill:
                bg_fill.pop(0)[1]()

    nc.compile()
    return nc


def _get_nc():
    if not _NC_CACHE:
        _NC_CACHE.append(_build())
    return _NC_CACHE[0]


def _to_np(a):
    try:
        return np.asarray(a)
    except Exception:
        import jax

        return np.asarray(jax.device_get(a))


def _prep_inputs(x, Wq, bq, Wk, bk, Wp, bp):
    import ml_dtypes

    bf16 = ml_dtypes.bfloat16
    x, Wq, bq, Wk, bk, Wp, bp = (_to_np(a) for a in (x, Wq, bq, Wk, bk, Wp, bp))
    wqt = np.ascontiguousarray(np.asarray(Wq, np.float32).T.astype(bf16))
    wkt = np.ascontiguousarray(np.asarray(Wk, np.float32).T.astype(bf16))
    wpt = np.ascontiguousarray(np.asarray(Wp, np.float32).T.astype(bf16))
    bqp = np.ascontiguousarray(np.asarray(bq, np.float32).reshape(FT, 128).T)
    bkp = np.ascontiguousarray(np.asarray(bk, np.float32).reshape(FT, 128).T)
    bpp = np.ascontiguousarray(
        np.broadcast_to(np.asarray(bp, np.float32)[None, :], (128, D))
    )
    base = {
        "wqt": wqt, "wkt": wkt, "wpt": wpt,
        "bqp": bqp, "bkp": bkp, "bpp": bpp,
    }
    in_maps = []
    xf = np.asarray(x, np.float32)
    for c in range(N_CORES):
        xc = np.ascontiguousarray(
            xf[BL * c : BL * (c + 1)].reshape(T, D).T.astype(bf16)
        )
        in_maps.append(dict(base, x=xc))
    return in_maps


def _run(in_maps, **kw):
    from concourse.bass_utils import run_bass_kernel_spmd

    nc = _get_nc()
    return run_bass_kernel_spmd(nc, in_maps, core_ids=list(range(N_CORES)), **kw)


def kernel(x, Wq, bq, Wk, bk, Wp, bp):
    in_maps = _prep_inputs(x, Wq, bq, Wk, bk, Wp, bp)
    res = _run(in_maps)
    out = np.concatenate(
        [r["out"].reshape(BL, N, D) for r in res.results], axis=0
    )
    return out.astype(np.float32)


# revision 46
# speedup vs baseline: 1.4087x; 1.0311x over previous
"""Trainium2 Bass kernel for nn_AttentionSeparateQKV (B=16, N=1024, D=768, H=12).

Data-parallel over batch: 8 NeuronCores x 2 batches each. Per core, per batch:
  Q/K projections (bf16 matmuls, feature-major PSUM, bias fused in the
  PSUM->SBUF copy on DVE).
  Scores computed TRANSPOSED: S^T[k, q] = (kT tile).T-matmul(qT), so the
  exp output (probs^T) is already in the AV-stationary orientation and no
  per-tile DMA transposes of probs are needed.
  exp on ACT with fused scale (1/sqrt(hd)) and range bias (-7.5*ln2); the
  bias cancels in normalization and keeps probs in fp8 range for the
  optional fp8 AV stage.
  V = K (reference reuses the K projection): v_ext = XBAR transpose of kT,
  laid out [128 keys, (kt, head) blocks, 80] with a ones column at 64 so
  the AV matmul also produces the softmax denominator for free.
  AV token-major: out[q, d] per head; normalization is a single DVE
  tensor_scalar divide by the denominator column (per-partition scalar).
  Out-projection from DMA-transposed out tiles; bias via DVE tensor_tensor.
"""

import sys

if "/opt/trn_rl_repo" not in sys.path:
    sys.path.insert(0, "/opt/trn_rl_repo")

import math

import numpy as np

B, N, D, H = 16, 1024, 768, 12
HD = D // H                # 64
N_CORES = 8
BL = B // N_CORES          # 2 batches per core
T = BL * N                 # 2048 tokens per core
FT = D // 128              # 6 feature tiles == head pairs
NKT = N // 128             # 8 key tiles per batch
NTT = N // 128             # 8 token tiles per batch
SCALE = float(HD) ** -0.5  # 0.125
CP = 7.5                   # probs = exp(SCALE*s - CP*ln2); cancels in softmax
EXP_BIAS = -CP * math.log(2.0)

PROBS_FP8 = True          # stage C switch: fp8e4 probs/v + DoubleRow AV

_NC_CACHE = []


def _build():
    import concourse.mybir as mybir
    import concourse.tile as tile
    from concourse import bacc

    F32 = mybir.dt.float32
    BF16 = mybir.dt.bfloat16
    FP8 = mybir.dt.float8e4
    EXP = mybir.ActivationFunctionType.Exp
    DIV = mybir.AluOpType.divide
    ADD = mybir.AluOpType.add
    DR = mybir.MatmulPerfMode.DoubleRow

    nc = bacc.Bacc("TRN2", target_bir_lowering=False, debug=False)

    x_d = nc.dram_tensor("x", [D, T], BF16, kind="ExternalInput").ap()
    wq_d = nc.dram_tensor("wqt", [D, D], BF16, kind="ExternalInput").ap()
    wk_d = nc.dram_tensor("wkt", [D, D], BF16, kind="ExternalInput").ap()
    wp_d = nc.dram_tensor("wpt", [D, D], BF16, kind="ExternalInput").ap()
    bq_d = nc.dram_tensor("bqp", [128, FT], F32, kind="ExternalInput").ap()
    bk_d = nc.dram_tensor("bkp", [128, FT], F32, kind="ExternalInput").ap()
    bp_d = nc.dram_tensor("bpp", [128, D], F32, kind="ExternalInput").ap()
    out_d = nc.dram_tensor("out", [T, D], F32, kind="ExternalOutput").ap()

    with tile.TileContext(nc) as tc:
        with (
            tc.tile_pool(name="const", bufs=1) as cpool,
            tc.tile_pool(name="xp", bufs=2) as xpool,
            tc.tile_pool(name="qk", bufs=2) as qkpool,
            tc.tile_pool(name="vx", bufs=2) as vpool,
            tc.tile_pool(name="probs", bufs=4) as prpool,
            tc.tile_pool(name="outsb", bufs=2) as opool,
            tc.tile_pool(name="outT", bufs=2) as otpool,
            tc.tile_pool(name="fin", bufs=4) as fpool,
            tc.tile_pool(name="rcp", bufs=8) as rpool,
            tc.tile_pool(name="ps_sc", bufs=2, space="PSUM") as ps_sc,
            tc.tile_pool(name="ps_sh", bufs=2, space="PSUM") as ps_sh,
        ):
            # ---- constants (small loads on gpsimd; bulk loads come first on
            # the shared DMA device via sync/scalar queues) ----
            bq_sb = cpool.tile([128, FT], F32, tag="bq")
            nc.gpsimd.dma_start(bq_sb[:], bq_d[:])
            bk_sb = cpool.tile([128, FT], F32, tag="bk")
            nc.gpsimd.dma_start(bk_sb[:], bk_d[:])
            bp_sb = cpool.tile([128, D], F32, tag="bp")
            nc.gpsimd.dma_start(bp_sb[:], bp_d[:])
            ebias = cpool.tile([128, 1], F32, tag="ebias")
            nc.vector.memset(ebias[:], EXP_BIAS)

            wq_sb = cpool.tile([128, FT, D], BF16, tag="wq")
            wk_sb = cpool.tile([128, FT, D], BF16, tag="wk")
            wp_sb = cpool.tile([128, FT, D], BF16, tag="wp")

            def emit_xT(b):
                xT = xpool.tile([128, FT, N], BF16, tag="xT", name="xT")
                nc.gpsimd.dma_start(
                    xT[:],
                    x_d[:, b * N : (b + 1) * N].rearrange("(ko kp) t -> kp ko t", kp=128),
                )
                return xT

            def emit_proj_piece(xT, f, half, wsb, bias_sb, dst):
                """One projection psum tile: feature tile f, token half `half`."""
                ps = ps_sh.tile([128, 512], F32, tag="sh", name="ps")
                for ks in range(FT):
                    nc.tensor.matmul(
                        ps[:],
                        wsb[:, ks, 128 * f : 128 * (f + 1)],
                        xT[:, ks, 512 * half : 512 * (half + 1)],
                        start=(ks == 0),
                        stop=(ks == FT - 1),
                    )
                nc.vector.tensor_scalar_add(
                    dst[:, 512 * half : 512 * (half + 1)], ps[:], bias_sb[:, f : f + 1]
                )

            def emit_vext(kTf):
                """v_ext[p, e, kt, d] = kTf[64e+d, 128kt+p]; col 64 = ones.

                One XBAR transpose per head half: in [64, 1024] -> out
                [128 key, 8 kt, 64 d] written into the 80-col padded layout.
                """
                vext = vpool.tile([128, 2, NKT, 80], BF16, tag="vext", name="vext")
                nc.gpsimd.memset(vext[:, :, :, 64:80], 0.0)
                nc.gpsimd.memset(vext[:, :, :, 64:65], 1.0)
                for e in range(2):
                    nc.sync.dma_start_transpose(
                        vext[:, e, :, 0:64], kTf[64 * e : 64 * (e + 1), :]
                    )
                if PROBS_FP8:
                    v8 = vpool.tile([128, 2, NKT, 80], FP8, tag="v8", name="v8")
                    nc.gpsimd.dma_start(v8[:], vext[:])  # casting DMA (SWDGE)
                    return (vext, v8)
                return (vext, None)

            def emit_outproj(outT, b, tt):
                fin = fpool.tile([128, D], F32, tag="fin", name="fin")
                for nh in range(2):
                    ps = ps_sh.tile([128, 512], F32, tag="sh", name="ps")
                    for ks in range(FT):
                        nc.tensor.matmul(
                            ps[:, 0:384],
                            outT[:, ks, 128 * tt : 128 * (tt + 1)],
                            wp_sb[:, ks, 384 * nh : 384 * (nh + 1)],
                            start=(ks == 0),
                            stop=(ks == FT - 1),
                        )
                    nc.vector.tensor_tensor(
                        fin[:, 384 * nh : 384 * (nh + 1)],
                        ps[:, 0:384],
                        bp_sb[:, 384 * nh : 384 * (nh + 1)],
                        ADD,
                    )
                nc.sync.dma_start(
                    out_d[b * N + 128 * tt : b * N + 128 * (tt + 1), :], fin[:]
                )

            # Filler queues of deferred PE work as (cost_cycles, fn) pairs,
            # consumed between exp emissions in sub-0.7us pieces so PE always
            # returns to scores production within ACT's consumption cadence.
            proj_fill = []
            bg_fill = []

            def pump(budget=1700):
                while budget > 0 and (proj_fill or bg_fill):
                    cost, fn = (proj_fill or bg_fill).pop(0)
                    fn()
                    budget -= cost

            def drain_proj():
                while proj_fill:
                    proj_fill.pop(0)[1]()

            def stage_proj(xT2, f2, half, wsb, bias_sb, dst):
                box = {}

                def sub_a():
                    ps = ps_sh.tile([128, 512], F32, tag="sh", name="ps")
                    box["ps"] = ps
                    for ks in range(3):
                        nc.tensor.matmul(
                            ps[:],
                            wsb[:, ks, 128 * f2 : 128 * (f2 + 1)],
                            xT2[:, ks, 512 * half : 512 * (half + 1)],
                            start=(ks == 0),
                            stop=False,
                        )

                def sub_b():
                    ps = box["ps"]
                    for ks in range(3, FT):
                        nc.tensor.matmul(
                            ps[:],
                            wsb[:, ks, 128 * f2 : 128 * (f2 + 1)],
                            xT2[:, ks, 512 * half : 512 * (half + 1)],
                            start=False,
                            stop=(ks == FT - 1),
                        )
                    nc.vector.tensor_scalar_add(
                        dst[:, 512 * half : 512 * (half + 1)], ps[:],
                        bias_sb[:, f2 : f2 + 1],
                    )

                proj_fill.append((1536, sub_a))
                proj_fill.append((1536, sub_b))

            def stage_outproj(out_sb, outT, b2, tt):
                nc.sync.dma_start_transpose(
                    outT[:, :, 128 * tt : 128 * (tt + 1)], out_sb[:, tt, :]
                )
                fbox = {}

                def fin_alloc():
                    fbox["fin"] = fpool.tile([128, D], F32, tag="fin", name="fin")

                def mk_sub(nh, part):
                    def sub():
                        if nh == 0 and part == 0:
                            fin_alloc()
                        if part == 0:
                            ps = ps_sh.tile([128, 512], F32, tag="sh", name="ps")
                            fbox["ps"] = ps
                        ps = fbox["ps"]
                        for ks in range(3 * part, 3 * part + 3):
                            nc.tensor.matmul(
                                ps[:, 0:384],
                                outT[:, ks, 128 * tt : 128 * (tt + 1)],
                                wp_sb[:, ks, 384 * nh : 384 * (nh + 1)],
                                start=(ks == 0),
                                stop=(ks == FT - 1),
                            )
                        if part == 1:
                            nc.vector.tensor_tensor(
                                fbox["fin"][:, 384 * nh : 384 * (nh + 1)],
                                ps[:, 0:384],
                                bp_sb[:, 384 * nh : 384 * (nh + 1)],
                                ADD,
                            )
                            if nh == 1:
                                nc.sync.dma_start(
                                    out_d[b2 * N + 128 * tt : b2 * N + 128 * (tt + 1), :],
                                    fbox["fin"][:],
                                )
                    return sub

                for nh in range(2):
                    for part in range(2):
                        bg_fill.append((1152, mk_sub(nh, part)))

            def flush_av(pend):
                """AV + normalize for one (head, chunk) unit."""
                pr, vpair, e, c, out_sb, outT, b2, h = pend[:8]
                tts = pend[8]
                vbf, v8 = vpair
                vext = v8 if pr.dtype != BF16 else vbf
                for tt in tts:
                    ttg = 4 * c + tt
                    av = ps_sh.tile([128, 512], F32, tag="sh", name="ps")
                    if pr.dtype != BF16:
                        for kp in range(NKT // 2):
                            nc.tensor.matmul(
                                av[:, 0:80],
                                pr[:, 2 * kp : 2 * kp + 2, 128 * tt : 128 * (tt + 1)],
                                vext[:, e, 2 * kp : 2 * kp + 2, :],
                                start=(kp == 0),
                                stop=(kp == NKT // 2 - 1),
                                perf_mode=DR,
                            )
                    else:
                        for kt in range(NKT):
                            nc.tensor.matmul(
                                av[:, 0:65],
                                pr[:, kt, 128 * tt : 128 * (tt + 1)],
                                vext[:, e, kt, 0:65],
                                start=(kt == 0),
                                stop=(kt == NKT - 1),
                            )
                    rcp = rpool.tile([128, 1], F32, tag="rcp", name="rcp")
                    nc.vector.reciprocal(rcp[:], av[:, 64:65])
                    nc.vector.tensor_scalar_mul(
                        out_sb[:, ttg, HD * h : HD * (h + 1)],
                        av[:, 0:64],
                        rcp[:, 0:1],
                    )
                if h == H - 1 and tts[-1] == 3:
                    # chunk c of batch b2 is complete: queue its out-projection
                    for tt in range(4 * c, 4 * c + 4):
                        stage_outproj(out_sb, outT, b2, tt)

            # ---- prologue (xT + wk first so K projection starts earliest) ----
            xT = xpool.tile([128, FT, N], BF16, tag="xT", name="xT")
            nc.sync.dma_start(
                xT[:], x_d[:, 0:N].rearrange("(ko kp) t -> kp ko t", kp=128)
            )
            wk_r = wk_d.rearrange("(ko kp) m -> kp ko m", kp=128)
            wq_r = wq_d.rearrange("(ko kp) m -> kp ko m", kp=128)
            nc.scalar.dma_start(wk_sb[:, :, 0:384], wk_r[:, :, 0:384])
            nc.scalar.dma_start(wq_sb[:, :, 0:384], wq_r[:, :, 0:384])
            nc.scalar.dma_start(wk_sb[:, :, 384:768], wk_r[:, :, 384:768])
            nc.scalar.dma_start(wq_sb[:, :, 384:768], wq_r[:, :, 384:768])
            # PE p-state warmup while the loads are in flight
            warm = cpool.tile([128, 512], BF16, tag="warm")
            nc.vector.memset(warm[:], 0.0)
            for i in range(22):
                ps = ps_sc.tile([128, 3, 512], F32, tag="sc", name="ps")
                nc.tensor.matmul(
                    ps[:, i % 3, :], warm[:, 0:128], warm[:], start=True, stop=True
                )
            qTf = qkpool.tile([128, N], BF16, tag="qTf", name="qTf")
            kTf = qkpool.tile([128, N], BF16, tag="kTf", name="kTf")
            for half in range(2):
                emit_proj_piece(xT, 0, half, wk_sb, bk_sb, kTf)
            emit_proj_piece(xT, 0, 0, wq_sb, bq_sb, qTf)
            vext = emit_vext(kTf)
            emit_proj_piece(xT, 0, 1, wq_sb, bq_sb, qTf)
            nc.scalar.dma_start(wp_sb[:], wp_d.rearrange("(ko kp) m -> kp ko m", kp=128))

            pend_av = []
            cur = (qTf, kTf, vext)
            xT_next = None

            for b in range(BL):
                if b > 0:
                    xT = xT_next
                out_sb = opool.tile([128, NTT, D], BF16, tag="out", name="out_sb")
                outT = otpool.tile([128, FT, N], BF16, tag="outT", name="outT")
                for f in range(FT):
                    qTf, kTf, vext = cur
                    # stage next projection (f+1, or next batch's f=0)
                    have_next = f < FT - 1 or b + 1 < BL
                    if f == 2 and b + 1 < BL:
                        xT_next = emit_xT(b + 1)
                    if have_next:
                        nf = (f + 1) % FT
                        nxt_x = xT if f < FT - 1 else xT_next
                        nxt_q = qkpool.tile([128, N], BF16, tag="qTf", name="qTf")
                        nxt_k = qkpool.tile([128, N], BF16, tag="kTf", name="kTf")
                        nxt_v = [None]
                        stage_proj(nxt_x, nf, 0, wk_sb, bk_sb, nxt_k)
                        stage_proj(nxt_x, nf, 1, wk_sb, bk_sb, nxt_k)
                        stage_proj(nxt_x, nf, 0, wq_sb, bq_sb, nxt_q)
                        proj_fill.append(
                            (100, lambda k2=nxt_k, v2=nxt_v: v2.__setitem__(0, emit_vext(k2)))
                        )
                        stage_proj(nxt_x, nf, 1, wq_sb, bq_sb, nxt_q)
                    for c in range(2):
                        for e in range(2):
                            h = 2 * f + e
                            use8 = PROBS_FP8 and not (b == 0 and f == 0)
                            pr = prpool.tile(
                                [128, NKT, 512], FP8 if use8 else BF16,
                                tag="pr", name="pr",
                            )
                            for k0, nk in ((0, 3), (3, 3), (6, 2)):
                                ps = ps_sc.tile([128, 3, 512], F32, tag="sc", name="ps")
                                for j in range(nk):
                                    kt = k0 + j
                                    nc.tensor.matmul(
                                        ps[:, j, :],
                                        kTf[64 * e : 64 * (e + 1), 128 * kt : 128 * (kt + 1)],
                                        qTf[64 * e : 64 * (e + 1), 512 * c : 512 * (c + 1)],
                                        start=True,
                                        stop=True,
                                    )
                                nc.scalar.activation(
                                    pr[:, k0 : k0 + nk, :],
                                    ps[:, 0:nk, :],
                                    EXP,
                                    scale=SCALE,
                                    bias=ebias[:],
                                )
                                budget = 2300 if have_next else 4600
                                if k0 == 0:
                                    pump(budget)
                                    if len(pend_av) >= 3:
                                        flush_av(pend_av.pop(0) + ((2, 3),))
                                elif k0 == 3:
                                    pump(budget)
                                    if len(pend_av) >= 2:
                                        flush_av(pend_av[0] + ((0, 1),))
                                        if len(pend_av) >= 3:
                                            pass
                            pend_av.append((pr, vext, e, c, out_sb, outT, b, h))
                    if have_next:
                        drain_proj()
                        cur = (nxt_q, nxt_k, nxt_v[0])
            done_half = set()
            for i_, p_ in enumerate(pend_av):
                if i_ == 0 and len(pend_av) == 3:
                    flush_av(p_ + ((2, 3),))
                else:
                    flush_av(p_ + ((0, 1),))
                    flush_av(p_ + ((2, 3),))
            while bg_fill:
                bg_fill.pop(0)[1]()

    nc.compile()
    return nc


def _get_nc():
    if not _NC_CACHE:
        _NC_CACHE.append(_build())
    return _NC_CACHE[0]


def _to_np(a):
    try:
        return np.asarray(a)
    except Exception:
        import jax

        return np.asarray(jax.device_get(a))


def _prep_inputs(x, Wq, bq, Wk, bk, Wp, bp):
    import ml_dtypes

    bf16 = ml_dtypes.bfloat16
    x, Wq, bq, Wk, bk, Wp, bp = (_to_np(a) for a in (x, Wq, bq, Wk, bk, Wp, bp))
    wqt = np.ascontiguousarray(np.asarray(Wq, np.float32).T.astype(bf16))
    wkt = np.ascontiguousarray(np.asarray(Wk, np.float32).T.astype(bf16))
    wpt = np.ascontiguousarray(np.asarray(Wp, np.float32).T.astype(bf16))
    bqp = np.ascontiguousarray(np.asarray(bq, np.float32).reshape(FT, 128).T)
    bkp = np.ascontiguousarray(np.asarray(bk, np.float32).reshape(FT, 128).T)
    bpp = np.ascontiguousarray(
        np.broadcast_to(np.asarray(bp, np.float32)[None, :], (128, D))
    )
    base = {
        "wqt": wqt, "wkt": wkt, "wpt": wpt,
        "bqp": bqp, "bkp": bkp, "bpp": bpp,
    }
    in_maps = []
    xf = np.asarray(x, np.float32)
    for c in range(N_CORES):
        xc = np.ascontiguousarray(
            xf[BL * c : BL * (c + 1)].reshape(T, D).T.astype(bf16)
        )
        in_maps.append(dict(base, x=xc))
    return in_maps


def _run(in_maps, **kw):
    from concourse.bass_utils import run_bass_kernel_spmd

    nc = _get_nc()
    return run_bass_kernel_spmd(nc, in_maps, core_ids=list(range(N_CORES)), **kw)


def kernel(x, Wq, bq, Wk, bk, Wp, bp):
    in_maps = _prep_inputs(x, Wq, bq, Wk, bk, Wp, bp)
    res = _run(in_maps)
    out = np.concatenate(
        [r["out"].reshape(BL, N, D) for r in res.results], axis=0
    )
    return out.astype(np.float32)
